# revision 9
# baseline (speedup 1.0000x reference)
"""Bass/Trainium2 kernel for nn_LoRAReins (ragged LoRA token attention).

Contract: kernel(**inputs) takes the FULL unsharded inputs (as produced by
reference.setup_inputs()) and returns the FULL [1025, 32, 1024] f32 output.

Strategy:
  - Host (numpy, f32): the tiny token/rank predictor MLPs (risky int-cast
    decisions need exact f32 CPU numerics), fold the rank/token masks into a
    per-sample masked-A matrix [R, T], fold `scale` into mt_w/mt_b, build the
    additive -1e9 softmax masks, convert all weights to bf16, and pre-
    transpose x to bf16 [B, C, N] so the device needs no PE transposes.
  - Device (8 NeuronCores, data-parallel over batch, 4 samples/core, no
    collectives): all the heavy matmuls in bf16 (attn logits, tokens_proj,
    delta, gate) + softmax + sigmoid + final f32 residual combine.
    Output error through the delta path is scaled by scale=1e-3, so bf16 is
    far inside tolerance (validated: max abs err ~1e-6 vs f32 reference).
"""

import os

import numpy as np
import ml_dtypes

BF = ml_dtypes.bfloat16

B_TOTAL = 32
N_TOK = 1024  # tokens after removing cls
C = 1024
T = 100
R = 16
N_CORES = 8
B_LOC = B_TOTAL // N_CORES
NT = N_TOK // 128  # 8 row tiles per sample
NCH = C // 128     # 8 contraction chunks


def _build_nc():
    import concourse.mybir as mybir
    import concourse.tile as tile
    from concourse import bacc
    from concourse.masks import make_identity
    from concourse.tile_rust import add_dep_helper

    f32 = mybir.dt.float32
    bf16 = mybir.dt.bfloat16
    AX = mybir.AxisListType
    AF = mybir.ActivationFunctionType

    nc = bacc.Bacc(None)
    x_d = nc.declare_dram_parameter("x", [B_LOC, N_TOK, C], f32, isOutput=False)
    xt_d = nc.declare_dram_parameter("xt", [B_LOC, C, N_TOK], bf16, isOutput=False)
    am_d = nc.declare_dram_parameter("amask", [R, B_LOC * T], bf16, isOutput=False)
    atm_d = nc.declare_dram_parameter("attnmask", [1, B_LOC * T], bf16, isOutput=False)
    bm_d = nc.declare_dram_parameter("bm", [R, C], bf16, isOutput=False)
    mtw_d = nc.declare_dram_parameter("mtw", [C, C], bf16, isOutput=False)
    mtb_d = nc.declare_dram_parameter("mtb", [1, C], bf16, isOutput=False)
    gw_d = nc.declare_dram_parameter("gw", [C, C], bf16, isOutput=False)
    gb_d = nc.declare_dram_parameter("gb", [1, C], bf16, isOutput=False)
    out_d = nc.declare_dram_parameter("out", [B_LOC, N_TOK, C], f32, isOutput=True)

    with tile.TileContext(nc) as tc:
        with (
            tc.tile_pool(name="const", bufs=1) as const,
            tc.tile_pool(name="xin", bufs=10) as xin,
            tc.tile_pool(name="xt", bufs=2) as xtp,
            tc.tile_pool(name="tokt", bufs=2) as toktp,
            tc.tile_pool(name="tp", bufs=2) as tpp,
            tc.tile_pool(name="gate", bufs=9) as gatep,
            tc.tile_pool(name="logit", bufs=9) as logitp,
            tc.tile_pool(name="soft", bufs=2) as softp,
            tc.tile_pool(name="og", bufs=2) as ogp,
            tc.tile_pool(name="ps_gate", bufs=4, space="PSUM") as ps_gate,
            tc.tile_pool(name="ps_dtp", bufs=2, space="PSUM") as ps_dtp,
            tc.tile_pool(name="ps_small", bufs=1, space="PSUM") as ps_small,
        ):
            # ---- constants / weights in SBUF ----
            ones = const.tile([1, 128], bf16, tag="ones")
            nc.gpsimd.memset(ones[:], 1.0)
            ident = const.tile([128, 128], bf16, tag="ident")
            make_identity(nc, ident[:])

            gw_sb = const.tile([128, NCH, C], bf16, tag="gw")
            nc.sync.dma_start(gw_sb[:], gw_d.rearrange("(a p) n -> p a n", p=128))
            mtw_sb = const.tile([128, NCH, C], bf16, tag="mtw")
            nc.sync.dma_start(mtw_sb[:], mtw_d.rearrange("(a p) n -> p a n", p=128))
            bm_sb = const.tile([R, C], bf16, tag="bm")
            nc.sync.dma_start(bm_sb[:], bm_d[:])
            mtb_sb = const.tile([1, C], bf16, tag="mtb")
            nc.sync.dma_start(mtb_sb[:], mtb_d[:])
            gb_sb = const.tile([1, C], bf16, tag="gb")
            nc.sync.dma_start(gb_sb[:], gb_d[:])
            am_sb = const.tile([R, B_LOC * T], bf16, tag="am")
            nc.sync.dma_start(am_sb[:], am_d[:])
            atm_sb = const.tile([1, B_LOC * T], bf16, tag="atm")
            nc.sync.dma_start(atm_sb[:], atm_d[:])

            prev_act = None  # last ACT instr of the previous phase
            for bi in range(B_LOC):
                # xT for the whole sample: 8 chunks of [c=128, n=1024] bf16,
                # contiguous 2KB partition lines straight from DRAM
                xT = xtp.tile([128, NCH, N_TOK], bf16, tag="xT")
                nc.sync.dma_start(
                    xT[:], xt_d[bi].rearrange("(a p) n -> p a n", p=128)
                )

                # tokensT[c, t] = Bm^T @ (A masked)  -> [C, T] bf16, 8 chunks
                tokT = toktp.tile([128, NCH * T], bf16, tag="tokT")
                for ci in range(NCH):
                    ps_tk = ps_small.tile([128, T], f32, tag="ps_small")
                    nc.tensor.matmul(
                        ps_tk[:],
                        bm_sb[:, ci * 128 : (ci + 1) * 128],
                        am_sb[:, bi * T : (bi + 1) * T],
                        start=True,
                        stop=True,
                    )
                    nc.vector.tensor_copy(
                        tokT[:, ci * T : (ci + 1) * T], ps_tk[:]
                    )

                # tokens_proj[t, c] = tokens @ mt_w + mt_b (scale folded in)
                tp_sb = tpp.tile([128, C], bf16, tag="tp")
                for j in range(2):
                    ps_tp = ps_dtp.tile([128, 512], f32, tag="ps_dtp")
                    for ci in range(NCH):
                        nc.tensor.matmul(
                            ps_tp[:T, :],
                            tokT[:, ci * T : (ci + 1) * T],
                            mtw_sb[:, ci, j * 512 : (j + 1) * 512],
                            start=(ci == 0),
                            stop=False,
                        )
                    nc.tensor.matmul(
                        ps_tp[:T, :],
                        ones[:1, :T],
                        mtb_sb[:1, j * 512 : (j + 1) * 512],
                        start=False,
                        stop=True,
                    )
                    nc.vector.tensor_copy(
                        tp_sb[:T, j * 512 : (j + 1) * 512], ps_tp[:T, :]
                    )

                # ---- phase A: all matmuls + sigmoids (ACT stays on the
                # sigmoid table), raw attn logits parked in SBUF ----
                xts, gates, logits = [], [], []
                for nt in range(NT):
                    x_t = xin.tile([128, C], f32, tag="x_t")
                    nc.sync.dma_start(
                        x_t[:], x_d[bi, nt * 128 : (nt + 1) * 128, :]
                    )

                    ps_g0 = ps_gate.tile([128, 512], f32, tag="ps_gate")
                    ps_g1 = ps_gate.tile([128, 512], f32, tag="ps_gate")
                    ps_a = ps_small.tile([128, T], f32, tag="ps_attn")
                    nsl = slice(nt * 128, (nt + 1) * 128)
                    for ci in range(NCH):
                        lhs = xT[:, ci, nsl]
                        nc.tensor.matmul(
                            ps_g0[:],
                            lhs,
                            gw_sb[:, ci, 0:512],
                            start=(ci == 0),
                            stop=False,
                        )
                        nc.tensor.matmul(
                            ps_g1[:],
                            lhs,
                            gw_sb[:, ci, 512:1024],
                            start=(ci == 0),
                            stop=False,
                        )
                        nc.tensor.matmul(
                            ps_a[:],
                            lhs,
                            tokT[:, ci * T : (ci + 1) * T],
                            start=(ci == 0),
                            stop=False,
                        )
                    # rank-1 accumulates: biases + additive softmax mask
                    nc.tensor.matmul(
                        ps_g0[:], ones[:1, :128], gb_sb[:1, 0:512],
                        start=False, stop=True,
                    )
                    nc.tensor.matmul(
                        ps_g1[:], ones[:1, :128], gb_sb[:1, 512:1024],
                        start=False, stop=True,
                    )
                    nc.tensor.matmul(
                        ps_a[:],
                        ones[:1, :128],
                        atm_sb[:1, bi * T : (bi + 1) * T],
                        start=False, stop=True,
                    )

                    gate_sb = gatep.tile([128, C], bf16, tag="gate")
                    s0 = nc.scalar.activation(gate_sb[:, 0:512], ps_g0[:], AF.Sigmoid)
                    # phase fence: keep ACT on one activation table per phase
                    # (each Sigmoid<->Exp switch costs a 1.28us table load)
                    if nt == 0 and prev_act is not None:
                        add_dep_helper(s0.ins, prev_act.ins, False,
                                       "ACT phase order: sigmoids after prev exp")
                    prev_act = nc.scalar.activation(
                        gate_sb[:, 512:1024], ps_g1[:], AF.Sigmoid
                    )

                    lg = logitp.tile([128, T], f32, tag="logit")
                    nc.vector.tensor_copy(lg[:], ps_a[:])
                    xts.append(x_t)
                    gates.append(gate_sb)
                    logits.append(lg)

                # ---- phase B: softmax (one Exp table load) + delta + out ----
                for nt in range(NT):
                    x_t, gate_sb, lg = xts[nt], gates[nt], logits[nt]
                    exp_sb = softp.tile([128, T], f32, tag="exp")
                    e0 = nc.scalar.activation(
                        exp_sb[:], lg[:], AF.Exp, scale=float(1.0 / 32.0)
                    )
                    if nt == 0:
                        add_dep_helper(e0.ins, prev_act.ins, False,
                                       "ACT phase order: exps after sigmoids")
                    prev_act = e0
                    ssum = softp.tile([128, 1], f32, tag="ssum")
                    nc.vector.reduce_sum(ssum[:], exp_sb[:], axis=AX.X)
                    rec = softp.tile([128, 1], f32, tag="rec")
                    nc.vector.reciprocal(rec[:], ssum[:])
                    attn16 = softp.tile([128, T], bf16, tag="attn16")
                    nc.vector.tensor_scalar_mul(attn16[:], exp_sb[:], rec[:, 0:1])

                    ps_at = ps_small.tile([128, 128], bf16, tag="ps_small")
                    nc.tensor.transpose(ps_at[:T, :], attn16[:], ident[:])
                    attnT = softp.tile([128, 128], bf16, tag="attnT")
                    nc.vector.tensor_copy(attnT[:T, :], ps_at[:T, :])
                    dg = ogp.tile([128, C], f32, tag="dg")
                    for j in range(2):
                        ps_d = ps_dtp.tile([128, 512], f32, tag="ps_dtp")
                        nc.tensor.matmul(
                            ps_d[:],
                            attnT[:T, :],
                            tp_sb[:T, j * 512 : (j + 1) * 512],
                            start=True,
                            stop=True,
                        )
                        nc.vector.tensor_mul(
                            dg[:, j * 512 : (j + 1) * 512],
                            ps_d[:],
                            gate_sb[:, j * 512 : (j + 1) * 512],
                        )
                    out_sb = ogp.tile([128, C], f32, tag="out_sb")
                    nc.vector.tensor_add(out_sb[:], dg[:], x_t[:])
                    nc.sync.dma_start(
                        out_d[bi, nt * 128 : (nt + 1) * 128, :], out_sb[:]
                    )

    nc.compile()
    return nc


_NC_CACHE = {}


def _get_nc():
    if "nc" not in _NC_CACHE:
        _NC_CACHE["nc"] = _build_nc()
    return _NC_CACHE["nc"]


def _host_prep(feats, layer, A_all, B_all, tlp_w1, tlp_b1, tlp_w2, tlp_b2,
               rp_w1, rp_b1, rp_w2, rp_b2, mt_w, mt_b, g_w, g_b, scale):
    """Predictor MLPs + mask folding, all in f32 numpy (matches jax CPU on the
    int-cast decisions with ~250x margin)."""
    f = np.float32
    feats = np.asarray(feats, f)
    x = feats[1:]                                     # [N, B, C]
    embed = x.mean(axis=0, dtype=f).astype(f)         # [B, C]

    def pred(w1, b1, w2, b2):
        h = np.maximum(embed @ np.asarray(w1, f) + np.asarray(b1, f), f(0))
        z = h.astype(f) @ np.asarray(w2, f) + np.asarray(b2, f)
        return (f(1) / (f(1) + np.exp(-z.astype(f))))[:, 0]

    token_ratios = f(1) / (f(1) + np.exp(-pred(tlp_w1, tlp_b1, tlp_w2, tlp_b2)))
    token_counts = np.clip((token_ratios * T).astype(np.int32), 1, T)
    ranks = np.clip((pred(rp_w1, rp_b1, rp_w2, rp_b2) * R).astype(np.int32), 1, R)

    rmask = (np.arange(R)[None, :] < ranks[:, None]).astype(f)     # [B, R]
    tmask = (np.arange(T)[None, :] < token_counts[:, None]).astype(f)  # [B, T]

    layer = int(np.asarray(layer))
    A = np.asarray(A_all, f)[layer]                   # [T, R]
    Bm = np.asarray(B_all, f)[layer]                  # [R, C]
    amask = (A.T[None, :, :] * rmask[:, :, None] * tmask[:, None, :]).astype(f)
    attnmask = ((f(1) - tmask) * f(-1e9)).astype(f)   # [B, T]
    sc = f(np.asarray(scale))
    return {
        "feats": feats,
        "amask": amask,
        "attnmask": attnmask,
        "bm": Bm.astype(BF),
        "mtw": (np.asarray(mt_w, f) * sc).astype(BF),
        "mtb": (np.asarray(mt_b, f) * sc).reshape(1, C).astype(BF),
        "gw": np.asarray(g_w, f).astype(BF),
        "gb": np.asarray(g_b, f).reshape(1, C).astype(BF),
    }


def make_in_maps(prep):
    feats = prep["feats"]
    in_maps = []
    for i in range(N_CORES):
        sl = slice(i * B_LOC, (i + 1) * B_LOC)
        xs = np.ascontiguousarray(feats[1:, sl, :].transpose(1, 0, 2))
        in_maps.append({
            "x": xs,
            "xt": np.ascontiguousarray(xs.transpose(0, 2, 1).astype(BF)),
            "amask": np.ascontiguousarray(
                prep["amask"][sl].transpose(1, 0, 2)
            ).reshape(R, B_LOC * T).astype(BF),
            "attnmask": prep["attnmask"][sl].reshape(1, B_LOC * T).astype(BF),
            "bm": prep["bm"],
            "mtw": prep["mtw"],
            "mtb": prep["mtb"],
            "gw": prep["gw"],
            "gb": prep["gb"],
        })
    return in_maps


def kernel(**inputs) -> np.ndarray:
    from concourse.bass_utils import run_bass_kernel_spmd

    prep = _host_prep(**inputs)
    in_maps = make_in_maps(prep)
    nc = _get_nc()
    trace = bool(int(os.environ.get("BASS_KERNEL_TRACE", "0")))
    res = run_bass_kernel_spmd(nc, in_maps, list(range(N_CORES)), trace=trace)
    if trace and res.exec_time_ns is not None:
        print(f"HW exec time: {res.exec_time_ns} ns")
        _NC_CACHE["exec_time_ns"] = res.exec_time_ns

    feats = prep["feats"]
    out = np.empty((N_TOK + 1, B_TOTAL, C), np.float32)
    out[0] = feats[0]
    for i in range(N_CORES):
        sl = slice(i * B_LOC, (i + 1) * B_LOC)
        out[1:, sl, :] = np.asarray(res.results[i]["out"]).transpose(1, 0, 2)
    return out
